# revision 10
# baseline (speedup 1.0000x reference)
"""Mixtral-style MoE block (T=2048, H=1024, F=2048, E=8, top-2) on 8 trn2
NeuronCores.

Expert-parallel: each core holds one expert's weights, computes the router
(replicated) + its expert's SwiGLU FFN over all tokens in fp32r, scales by
the renormalized top-2 combine weight for its expert, and per-T-half
ReduceScatter(add) collectives combine the partial outputs (the first RS
overlaps the second half's compute); the host reassembles the shards.
"""
import numpy as np

try:
    import concourse  # noqa: F401
except ImportError:  # pragma: no cover
    import sys
    sys.path.insert(0, "/opt/trn_rl_repo")

from concourse import mybir, bacc
import concourse.tile as tile
from concourse.masks import make_identity
from concourse.bass_utils import run_bass_kernel_spmd

T, H, F, E, TOP_K = 2048, 1024, 2048, 8, 2
P = 128
NCHUNK = T // P      # 16 token chunks
KH = H // P          # 8 k-tiles over H
KF = F // P          # 16 k-tiles over F
NHALF = 2            # T processed in halves (SBUF capacity)
TH = T // NHALF      # 1024 tokens per half
MH = TH // P         # 8 m-tiles per half
MG = 4               # phase-B m-tiles per PSUM group (MG*2 banks)
F32 = mybir.dt.float32
F32R = mybir.dt.float32r
PSUM = "PSUM"

_NC_CACHE = {}


def _router(nc, tc, small, xt_s, gw_s, esel_s, ident):
    """Replicated router: returns c_e [P, NCHUNK, 1] — this core's expert
    combine weight per token (token t = c*128 + p)."""
    with tc.tile_pool(name="psR", bufs=1, space=PSUM) as psR:
        logits_s = small.tile([E, T], F32)
        for n in range(T // 512):
            ps = psR.tile([E, 512], F32, tag="ps_log")
            for k in range(KH):
                nc.tensor.matmul(ps[:], lhsT=gw_s[:, k, :],
                                 rhs=xt_s[n][:, k, :],
                                 start=(k == 0), stop=(k == KH - 1))
            nc.vector.tensor_copy(logits_s[:, n * 512:(n + 1) * 512], ps[:])

        lt_ps = psR.tile([P, NCHUNK * E], F32, tag="ps_tr")
        for c in range(NCHUNK):
            nc.tensor.transpose(out=lt_ps[:, c * E:(c + 1) * E],
                                in_=logits_s[:, c * P:(c + 1) * P],
                                identity=ident[:E, :E])
        lg = small.tile([P, NCHUNK, E], F32)
        nc.vector.tensor_copy(lg[:],
                              lt_ps[:].rearrange("p (c e) -> p c e", e=E))

    bc = [P, NCHUNK, E]
    m1 = small.tile([P, NCHUNK, 1], F32)
    nc.vector.reduce_max(m1[:], lg[:], axis=mybir.AxisListType.X)
    ls = small.tile([P, NCHUNK, E], F32)
    nc.vector.tensor_tensor(ls[:], lg[:], m1[:].to_broadcast(bc),
                            op=mybir.AluOpType.subtract)
    mask1 = small.tile([P, NCHUNK, E], F32)
    nc.vector.tensor_scalar(mask1[:], ls[:], 0.0, None,
                            op0=mybir.AluOpType.is_ge)
    masked = small.tile([P, NCHUNK, E], F32)
    nc.vector.scalar_tensor_tensor(out=masked[:], in0=mask1[:], scalar=-1e30,
                                   in1=ls[:], op0=mybir.AluOpType.mult,
                                   op1=mybir.AluOpType.add)
    m2 = small.tile([P, NCHUNK, 1], F32)
    nc.vector.reduce_max(m2[:], masked[:], axis=mybir.AxisListType.X)
    mask12 = small.tile([P, NCHUNK, E], F32)
    nc.vector.tensor_tensor(mask12[:], ls[:], m2[:].to_broadcast(bc),
                            op=mybir.AluOpType.is_ge)
    ex = small.tile([P, NCHUNK, E], F32)
    nc.scalar.activation(ex[:], ls[:], mybir.ActivationFunctionType.Exp)
    wun = small.tile([P, NCHUNK, E], F32)
    nc.vector.tensor_tensor(wun[:], ex[:], mask12[:], op=mybir.AluOpType.mult)
    den = small.tile([P, NCHUNK, 1], F32)
    nc.vector.reduce_sum(den[:], wun[:], axis=mybir.AxisListType.X)
    rden = small.tile([P, NCHUNK, 1], F32)
    nc.vector.reciprocal(rden[:], den[:])
    cw = small.tile([P, NCHUNK, E], F32)
    nc.vector.tensor_tensor(cw[:], wun[:],
                            esel_s[:].unsqueeze(1).to_broadcast(bc),
                            op=mybir.AluOpType.mult)
    cwn = small.tile([P, NCHUNK, E], F32)
    nc.vector.tensor_tensor(cwn[:], cw[:], rden[:].to_broadcast(bc),
                            op=mybir.AluOpType.mult)
    c_e = small.tile([P, NCHUNK, 1], F32)
    nc.vector.reduce_sum(c_e[:], cwn[:], axis=mybir.AxisListType.X)
    return c_e


def _phase_a(nc, psA, wpool, evac, xt_s, inter, w1v, w3v, t0):
    """interT[f, t] = silu(w1.T x) * (w3.T x) for tokens [t0, t0+TH)."""
    for f in range(KF):
        w1f = wpool.tile([P, KH, P], F32R, tag="w1f", name="w1f")
        nc.gpsimd.dma_start(out=w1f[:], in_=w1v[:, :, f * P:(f + 1) * P])
        w3f = wpool.tile([P, KH, P], F32R, tag="w3f", name="w3f")
        nc.gpsimd.dma_start(out=w3f[:], in_=w3v[:, :, f * P:(f + 1) * P])
        for n in range(TH // 512):
            xtn = xt_s[(t0 + n * 512) // 512]
            fs = slice(n * 512, (n + 1) * 512)
            ps1 = psA.tile([P, 512], F32, tag="ps1", name="ps1")
            for k in range(KH):
                nc.tensor.matmul(ps1[:], lhsT=w1f[:, k, :], rhs=xtn[:, k, :],
                                 start=(k == 0), stop=(k == KH - 1))
            ps3 = psA.tile([P, 512], F32, tag="ps3", name="ps3")
            for k in range(KH):
                nc.tensor.matmul(ps3[:], lhsT=w3f[:, k, :], rhs=xtn[:, k, :],
                                 start=(k == 0), stop=(k == KH - 1))
            sil = evac.tile([P, 512], F32, tag="sil", name="sil")
            nc.scalar.activation(sil[:], ps1[:],
                                 mybir.ActivationFunctionType.Silu)
            nc.vector.tensor_tensor(inter[:, f, fs], sil[:], ps3[:],
                                    op=mybir.AluOpType.mult)


def _phase_b(nc, psB, wpool, evac, inter, w2v, c_e, cc_q, th):
    """cc_q[g][t, :] = (interT.T @ w2) * c_e for this T-half's quarters."""
    for g in range(MH // MG):
        cc_in = cc_q[g]
        psbs = [[psB.tile([P, 512], F32, tag=f"psb{m}{n}", name=f"psb{m}{n}")
                 for n in range(H // 512)] for m in range(MG)]
        for k in range(KF):
            w2k = wpool.tile([P, H], F32R, tag="w2k", name="w2k", bufs=4)
            nc.scalar.dma_start(out=w2k[:], in_=w2v[:, k, :])
            for m in range(MG):
                ma = g * MG + m
                for n in range(H // 512):
                    nc.tensor.matmul(psbs[m][n][:],
                                     lhsT=inter[:, k, ma * P:(ma + 1) * P],
                                     rhs=w2k[:, n * 512:(n + 1) * 512],
                                     start=(k == 0), stop=(k == KF - 1))
        for m in range(MG):
            ma = g * MG + m
            for n in range(H // 512):
                o = evac.tile([P, 512], F32, tag="o", name="o")
                nc.vector.tensor_scalar_mul(o[:], psbs[m][n][:],
                                            c_e[:, th * MH + ma, :])
                nc.sync.dma_start(
                    out=cc_in.ap()[m * P:(m + 1) * P,
                                   n * 512:(n + 1) * 512],
                    in_=o[:])


def build():
    nc = bacc.Bacc("TRN2", target_bir_lowering=False, debug=False,
                   num_devices=E)
    xt = nc.dram_tensor("xt", [H, T], F32R, kind="ExternalInput")
    gw = nc.dram_tensor("gw", [H, E], F32R, kind="ExternalInput")
    esel = nc.dram_tensor("esel", [P, E], F32, kind="ExternalInput")
    w1 = nc.dram_tensor("w1", [H, F], F32R, kind="ExternalInput")
    w3 = nc.dram_tensor("w3", [H, F], F32R, kind="ExternalInput")
    w2 = nc.dram_tensor("w2", [F, H], F32R, kind="ExternalInput")
    out_shard = nc.dram_tensor("out_shard", [2 * P, H], F32,
                               kind="ExternalOutput")

    NQ = 4
    TQ = T // NQ  # 512 tokens per quarter
    cc_in = [nc.dram_tensor(f"cc_in{i}", [TQ, H], F32, kind="Internal")
             for i in range(NQ)]
    cc_out = [nc.dram_tensor(f"cc_out{i}", [TQ // E, H], F32, kind="Internal")
              for i in range(NQ)]

    with tile.TileContext(nc) as tc:
        with (
            tc.tile_pool(name="big", bufs=1) as big,
            tc.tile_pool(name="small", bufs=1) as small,
            tc.tile_pool(name="wpool", bufs=2) as wpool,
            tc.tile_pool(name="evac", bufs=3) as evac,
        ):
            xtv = xt.ap().rearrange("(k p) t -> p k t", p=P)
            xt_s = []
            for n in range(T // 512):  # separate tiles so compute starts early
                xtn = big.tile([P, KH, 512], F32R, name=f"xt{n}")
                eng = nc.gpsimd if n == 0 else nc.sync
                eng.dma_start(out=xtn[:],
                              in_=xtv[:, :, n * 512:(n + 1) * 512])
                xt_s.append(xtn)
            inter = big.tile([P, KF, TH], F32R)  # interT for current half

            gw_s = small.tile([P, KH, E], F32R)
            nc.sync.dma_start(out=gw_s[:],
                              in_=gw.ap().rearrange("(k p) e -> p k e", p=P))
            esel_s = small.tile([P, E], F32)
            nc.sync.dma_start(out=esel_s[:], in_=esel.ap())
            ident = small.tile([P, P], F32)
            make_identity(nc, ident[:])

            w1v = w1.ap().rearrange("(k p) f -> p k f", p=P)
            w3v = w3.ap().rearrange("(k p) f -> p k f", p=P)
            w2v = w2.ap().rearrange("(k p) h -> p k h", p=P)

            # half 0 phase A first so PE starts as soon as xt chunk 0 lands
            with tc.tile_pool(name="psA0", bufs=2, space=PSUM) as psA:
                _phase_a(nc, psA, wpool, evac, xt_s, inter, w1v, w3v, 0)
            c_e = _router(nc, tc, small, xt_s, gw_s, esel_s, ident)
            def rs(q):
                nc.gpsimd.collective_compute(
                    "ReduceScatter", mybir.AluOpType.add,
                    replica_groups=[list(range(E))],
                    ins=[cc_in[q].ap()], outs=[cc_out[q].ap()])

            with tc.tile_pool(name="psB0", bufs=1, space=PSUM) as psB:
                _phase_b(nc, psB, wpool, evac, inter, w2v, c_e, cc_in[0:2], 0)
            with tc.tile_pool(name="psA1", bufs=2, space=PSUM) as psA:
                _phase_a(nc, psA, wpool, evac, xt_s, inter, w1v, w3v, TH)
            rs(0)
            rs(1)
            with tc.tile_pool(name="psB1", bufs=1, space=PSUM) as psB:
                _phase_b(nc, psB, wpool, evac, inter, w2v, c_e, cc_in[2:4], 1)
            rs(2)
            rs(3)

            TQ8 = (T // 4) // E
            for q in range(4):
                nc.sync.dma_start(
                    out=out_shard.ap()[q * TQ8:(q + 1) * TQ8, :],
                    in_=cc_out[q].ap())
    nc.compile()
    return nc


def kernel(hidden_states, gate_w, w1, w2, w3):
    if "nc" not in _NC_CACHE:
        _NC_CACHE["nc"] = build()
    nc = _NC_CACHE["nc"]

    res = run_bass_kernel_spmd(nc, make_in_maps(hidden_states, gate_w, w1, w2, w3),
                               core_ids=list(range(E)), trace=False)
    return assemble(res.results)


def make_in_maps(hidden_states, gate_w, w1, w2, w3):
    xt = np.ascontiguousarray(hidden_states.T)
    in_maps = []
    for e in range(E):
        sel = np.zeros((P, E), dtype=np.float32)
        sel[:, e] = 1.0
        in_maps.append({
            "xt": xt,
            "gw": np.ascontiguousarray(gate_w),
            "esel": sel,
            "w1": np.ascontiguousarray(w1[e]),
            "w3": np.ascontiguousarray(w3[e]),
            "w2": np.ascontiguousarray(w2[e]),
        })
    return in_maps


def assemble(results):
    out = np.empty((T, H), dtype=np.float32)
    tq = T // 4
    rq = tq // E  # 64 rows per core per quarter
    for r in range(E):
        sh = results[r]["out_shard"]
        for q in range(4):
            t0 = q * tq + r * rq
            out[t0:t0 + rq] = sh[q * rq:(q + 1) * rq]
    return out


# revision 11
# speedup vs baseline: 1.0344x; 1.0344x over previous
"""Mixtral-style MoE block (T=2048, H=1024, F=2048, E=8, top-2) on 8 trn2
NeuronCores.

Expert-parallel: each core holds one expert's weights, computes the router
(replicated) + its expert's SwiGLU FFN over all tokens in fp32r, scales by
the renormalized top-2 combine weight for its expert, and per-T-half
ReduceScatter(add) collectives combine the partial outputs (the first RS
overlaps the second half's compute); the host reassembles the shards.
"""
import numpy as np

try:
    import concourse  # noqa: F401
except ImportError:  # pragma: no cover
    import sys
    sys.path.insert(0, "/opt/trn_rl_repo")

from concourse import mybir, bacc
import concourse.tile as tile
from concourse.masks import make_identity
from concourse.bass_utils import run_bass_kernel_spmd

T, H, F, E, TOP_K = 2048, 1024, 2048, 8, 2
P = 128
NCHUNK = T // P      # 16 token chunks
KH = H // P          # 8 k-tiles over H
KF = F // P          # 16 k-tiles over F
NHALF = 2            # T processed in halves (SBUF capacity)
TH = T // NHALF      # 1024 tokens per half
MH = TH // P         # 8 m-tiles per half
MG = 4               # phase-B m-tiles per PSUM group (MG*2 banks)
F32 = mybir.dt.float32
F32R = mybir.dt.float32r
PSUM = "PSUM"

_NC_CACHE = {}


def _router(nc, tc, small, xt_s, gw_s, esel_s, ident):
    """Replicated router: returns c_e [P, NCHUNK, 1] — this core's expert
    combine weight per token (token t = c*128 + p)."""
    with tc.tile_pool(name="psR", bufs=1, space=PSUM) as psR:
        logits_s = small.tile([E, T], F32)
        for n in range(T // 512):
            ps = psR.tile([E, 512], F32, tag="ps_log")
            for k in range(KH):
                nc.tensor.matmul(ps[:], lhsT=gw_s[:, k, :],
                                 rhs=xt_s[n][:, k, :],
                                 start=(k == 0), stop=(k == KH - 1))
            nc.vector.tensor_copy(logits_s[:, n * 512:(n + 1) * 512], ps[:])

        lt_ps = psR.tile([P, NCHUNK * E], F32, tag="ps_tr")
        for c in range(NCHUNK):
            nc.tensor.transpose(out=lt_ps[:, c * E:(c + 1) * E],
                                in_=logits_s[:, c * P:(c + 1) * P],
                                identity=ident[:E, :E])
        lg = small.tile([P, NCHUNK, E], F32)
        nc.vector.tensor_copy(lg[:],
                              lt_ps[:].rearrange("p (c e) -> p c e", e=E))

    bc = [P, NCHUNK, E]
    m1 = small.tile([P, NCHUNK, 1], F32)
    nc.vector.reduce_max(m1[:], lg[:], axis=mybir.AxisListType.X)
    ls = small.tile([P, NCHUNK, E], F32)
    nc.vector.tensor_tensor(ls[:], lg[:], m1[:].to_broadcast(bc),
                            op=mybir.AluOpType.subtract)
    mask1 = small.tile([P, NCHUNK, E], F32)
    nc.vector.tensor_scalar(mask1[:], ls[:], 0.0, None,
                            op0=mybir.AluOpType.is_ge)
    masked = small.tile([P, NCHUNK, E], F32)
    nc.vector.scalar_tensor_tensor(out=masked[:], in0=mask1[:], scalar=-1e30,
                                   in1=ls[:], op0=mybir.AluOpType.mult,
                                   op1=mybir.AluOpType.add)
    m2 = small.tile([P, NCHUNK, 1], F32)
    nc.vector.reduce_max(m2[:], masked[:], axis=mybir.AxisListType.X)
    mask12 = small.tile([P, NCHUNK, E], F32)
    nc.vector.tensor_tensor(mask12[:], ls[:], m2[:].to_broadcast(bc),
                            op=mybir.AluOpType.is_ge)
    ex = small.tile([P, NCHUNK, E], F32)
    nc.scalar.activation(ex[:], ls[:], mybir.ActivationFunctionType.Exp)
    wun = small.tile([P, NCHUNK, E], F32)
    nc.vector.tensor_tensor(wun[:], ex[:], mask12[:], op=mybir.AluOpType.mult)
    den = small.tile([P, NCHUNK, 1], F32)
    nc.vector.reduce_sum(den[:], wun[:], axis=mybir.AxisListType.X)
    rden = small.tile([P, NCHUNK, 1], F32)
    nc.vector.reciprocal(rden[:], den[:])
    cw = small.tile([P, NCHUNK, E], F32)
    nc.vector.tensor_tensor(cw[:], wun[:],
                            esel_s[:].unsqueeze(1).to_broadcast(bc),
                            op=mybir.AluOpType.mult)
    cwn = small.tile([P, NCHUNK, E], F32)
    nc.vector.tensor_tensor(cwn[:], cw[:], rden[:].to_broadcast(bc),
                            op=mybir.AluOpType.mult)
    c_e = small.tile([P, NCHUNK, 1], F32)
    nc.vector.reduce_sum(c_e[:], cwn[:], axis=mybir.AxisListType.X)
    return c_e


def _phase_a(nc, psA, wpool, evac, xt_s, inter, w1v, w3v, t0):
    """interT[f, t] = silu(w1.T x) * (w3.T x) for tokens [t0, t0+TH)."""
    for f in range(KF):
        w1f = wpool.tile([P, KH, P], F32R, tag="w1f", name="w1f")
        nc.gpsimd.dma_start(out=w1f[:], in_=w1v[:, :, f * P:(f + 1) * P])
        w3f = wpool.tile([P, KH, P], F32R, tag="w3f", name="w3f")
        nc.gpsimd.dma_start(out=w3f[:], in_=w3v[:, :, f * P:(f + 1) * P])
        for n in range(TH // 512):
            xtn = xt_s[(t0 + n * 512) // 512]
            fs = slice(n * 512, (n + 1) * 512)
            ps1 = psA.tile([P, 512], F32, tag="ps1", name="ps1")
            for k in range(KH):
                nc.tensor.matmul(ps1[:], lhsT=w1f[:, k, :], rhs=xtn[:, k, :],
                                 start=(k == 0), stop=(k == KH - 1))
            ps3 = psA.tile([P, 512], F32, tag="ps3", name="ps3")
            for k in range(KH):
                nc.tensor.matmul(ps3[:], lhsT=w3f[:, k, :], rhs=xtn[:, k, :],
                                 start=(k == 0), stop=(k == KH - 1))
            sil = evac.tile([P, 512], F32, tag="sil", name="sil")
            nc.scalar.activation(sil[:], ps1[:],
                                 mybir.ActivationFunctionType.Silu)
            nc.vector.tensor_tensor(inter[:, f, fs], sil[:], ps3[:],
                                    op=mybir.AluOpType.mult)


def _phase_b(nc, psB, wpool, evac, inter, w2v, c_e, cc_q, th):
    """cc_q[g][t, :] = (interT.T @ w2) * c_e for this T-half's quarters."""
    for g in range(MH // MG):
        cc_in = cc_q[g]
        psbs = [[psB.tile([P, 512], F32, tag=f"psb{m}{n}", name=f"psb{m}{n}")
                 for n in range(H // 512)] for m in range(MG)]
        for k in range(KF):
            w2k = wpool.tile([P, H], F32R, tag="w2k", name="w2k", bufs=4)
            nc.gpsimd.dma_start(out=w2k[:], in_=w2v[:, k, :])
            for m in range(MG):
                ma = g * MG + m
                for n in range(H // 512):
                    nc.tensor.matmul(psbs[m][n][:],
                                     lhsT=inter[:, k, ma * P:(ma + 1) * P],
                                     rhs=w2k[:, n * 512:(n + 1) * 512],
                                     start=(k == 0), stop=(k == KF - 1))
        for m in range(MG):
            ma = g * MG + m
            for n in range(H // 512):
                o = evac.tile([P, 512], F32, tag="o", name="o")
                nc.vector.tensor_scalar_mul(o[:], psbs[m][n][:],
                                            c_e[:, th * MH + ma, :])
                nc.sync.dma_start(
                    out=cc_in.ap()[m * P:(m + 1) * P,
                                   n * 512:(n + 1) * 512],
                    in_=o[:])


def build():
    nc = bacc.Bacc("TRN2", target_bir_lowering=False, debug=False,
                   num_devices=E)
    xt = nc.dram_tensor("xt", [H, T], F32R, kind="ExternalInput")
    gw = nc.dram_tensor("gw", [H, E], F32R, kind="ExternalInput")
    esel = nc.dram_tensor("esel", [P, E], F32, kind="ExternalInput")
    w1 = nc.dram_tensor("w1", [H, F], F32R, kind="ExternalInput")
    w3 = nc.dram_tensor("w3", [H, F], F32R, kind="ExternalInput")
    w2 = nc.dram_tensor("w2", [F, H], F32R, kind="ExternalInput")
    out_shard = nc.dram_tensor("out_shard", [2 * P, H], F32,
                               kind="ExternalOutput")

    NQ = 4
    TQ = T // NQ  # 512 tokens per quarter
    cc_in = [nc.dram_tensor(f"cc_in{i}", [TQ, H], F32, kind="Internal")
             for i in range(NQ)]
    cc_out = [nc.dram_tensor(f"cc_out{i}", [TQ // E, H], F32, kind="Internal")
              for i in range(NQ)]

    with tile.TileContext(nc) as tc:
        with (
            tc.tile_pool(name="big", bufs=1) as big,
            tc.tile_pool(name="small", bufs=1) as small,
            tc.tile_pool(name="wpool", bufs=2) as wpool,
            tc.tile_pool(name="evac", bufs=3) as evac,
        ):
            xtv = xt.ap().rearrange("(k p) t -> p k t", p=P)
            xt_s = []
            for n in range(T // 512):  # separate tiles so compute starts early
                xtn = big.tile([P, KH, 512], F32R, name=f"xt{n}")
                eng = nc.gpsimd if n == 0 else nc.sync
                eng.dma_start(out=xtn[:],
                              in_=xtv[:, :, n * 512:(n + 1) * 512])
                xt_s.append(xtn)
            inter = big.tile([P, KF, TH], F32R)  # interT for current half

            gw_s = small.tile([P, KH, E], F32R)
            nc.sync.dma_start(out=gw_s[:],
                              in_=gw.ap().rearrange("(k p) e -> p k e", p=P))
            esel_s = small.tile([P, E], F32)
            nc.sync.dma_start(out=esel_s[:], in_=esel.ap())
            ident = small.tile([P, P], F32)
            make_identity(nc, ident[:])

            w1v = w1.ap().rearrange("(k p) f -> p k f", p=P)
            w3v = w3.ap().rearrange("(k p) f -> p k f", p=P)
            w2v = w2.ap().rearrange("(k p) h -> p k h", p=P)

            # half 0 phase A first so PE starts as soon as xt chunk 0 lands
            with tc.tile_pool(name="psA0", bufs=2, space=PSUM) as psA:
                _phase_a(nc, psA, wpool, evac, xt_s, inter, w1v, w3v, 0)
            c_e = _router(nc, tc, small, xt_s, gw_s, esel_s, ident)
            def rs(q):
                nc.gpsimd.collective_compute(
                    "ReduceScatter", mybir.AluOpType.add,
                    replica_groups=[list(range(E))],
                    ins=[cc_in[q].ap()], outs=[cc_out[q].ap()])

            with tc.tile_pool(name="psB0", bufs=1, space=PSUM) as psB:
                _phase_b(nc, psB, wpool, evac, inter, w2v, c_e, cc_in[0:2], 0)
            rs(0)
            with tc.tile_pool(name="psA1", bufs=2, space=PSUM) as psA:
                _phase_a(nc, psA, wpool, evac, xt_s, inter, w1v, w3v, TH)
            rs(1)
            with tc.tile_pool(name="psB1", bufs=1, space=PSUM) as psB:
                _phase_b(nc, psB, wpool, evac, inter, w2v, c_e, cc_in[2:4], 1)
            rs(2)
            rs(3)

            TQ8 = (T // 4) // E
            for q in range(4):
                nc.sync.dma_start(
                    out=out_shard.ap()[q * TQ8:(q + 1) * TQ8, :],
                    in_=cc_out[q].ap())
    nc.compile()
    return nc


def kernel(hidden_states, gate_w, w1, w2, w3):
    if "nc" not in _NC_CACHE:
        _NC_CACHE["nc"] = build()
    nc = _NC_CACHE["nc"]

    res = run_bass_kernel_spmd(nc, make_in_maps(hidden_states, gate_w, w1, w2, w3),
                               core_ids=list(range(E)), trace=False)
    return assemble(res.results)


def make_in_maps(hidden_states, gate_w, w1, w2, w3):
    xt = np.ascontiguousarray(hidden_states.T)
    in_maps = []
    for e in range(E):
        sel = np.zeros((P, E), dtype=np.float32)
        sel[:, e] = 1.0
        in_maps.append({
            "xt": xt,
            "gw": np.ascontiguousarray(gate_w),
            "esel": sel,
            "w1": np.ascontiguousarray(w1[e]),
            "w3": np.ascontiguousarray(w3[e]),
            "w2": np.ascontiguousarray(w2[e]),
        })
    return in_maps


def assemble(results):
    out = np.empty((T, H), dtype=np.float32)
    tq = T // 4
    rq = tq // E  # 64 rows per core per quarter
    for r in range(E):
        sh = results[r]["out_shard"]
        for q in range(4):
            t0 = q * tq + r * rq
            out[t0:t0 + rq] = sh[q * rq:(q + 1) * rq]
    return out


# revision 12
# speedup vs baseline: 1.0680x; 1.0325x over previous
"""Mixtral-style MoE block (T=2048, H=1024, F=2048, E=8, top-2) on 8 trn2
NeuronCores.

Expert-parallel: each core holds one expert's weights, computes the router
(replicated) + its expert's SwiGLU FFN over all tokens in fp32r, scales by
the renormalized top-2 combine weight for its expert, and per-T-half
ReduceScatter(add) collectives combine the partial outputs (the first RS
overlaps the second half's compute); the host reassembles the shards.
"""
import numpy as np

try:
    import concourse  # noqa: F401
except ImportError:  # pragma: no cover
    import sys
    sys.path.insert(0, "/opt/trn_rl_repo")

from concourse import mybir, bacc
import concourse.tile as tile
from concourse.masks import make_identity
from concourse.bass_utils import run_bass_kernel_spmd

T, H, F, E, TOP_K = 2048, 1024, 2048, 8, 2
P = 128
NCHUNK = T // P      # 16 token chunks
KH = H // P          # 8 k-tiles over H
KF = F // P          # 16 k-tiles over F
NHALF = 2            # T processed in halves (SBUF capacity)
TH = T // NHALF      # 1024 tokens per half
MH = TH // P         # 8 m-tiles per half
MG = 4               # phase-B m-tiles per PSUM group (MG*2 banks)
F32 = mybir.dt.float32
F32R = mybir.dt.float32r
PSUM = "PSUM"

_NC_CACHE = {}


def _router(nc, tc, small, xt_s, gw_s, esel_s, ident):
    """Replicated router: returns c_e [P, NCHUNK, 1] — this core's expert
    combine weight per token (token t = c*128 + p)."""
    with tc.tile_pool(name="psR", bufs=1, space=PSUM) as psR:
        logits_s = small.tile([E, T], F32)
        for n in range(T // 512):
            ps = psR.tile([E, 512], F32, tag="ps_log")
            for k in range(KH):
                nc.tensor.matmul(ps[:], lhsT=gw_s[:, k, :],
                                 rhs=xt_s[n][:, k, :],
                                 start=(k == 0), stop=(k == KH - 1))
            nc.vector.tensor_copy(logits_s[:, n * 512:(n + 1) * 512], ps[:])

        lt_ps = psR.tile([P, NCHUNK * E], F32, tag="ps_tr")
        for c in range(NCHUNK):
            nc.tensor.transpose(out=lt_ps[:, c * E:(c + 1) * E],
                                in_=logits_s[:, c * P:(c + 1) * P],
                                identity=ident[:E, :E])
        lg = small.tile([P, NCHUNK, E], F32)
        nc.vector.tensor_copy(lg[:],
                              lt_ps[:].rearrange("p (c e) -> p c e", e=E))

    bc = [P, NCHUNK, E]
    m1 = small.tile([P, NCHUNK, 1], F32)
    nc.vector.reduce_max(m1[:], lg[:], axis=mybir.AxisListType.X)
    ls = small.tile([P, NCHUNK, E], F32)
    nc.vector.tensor_tensor(ls[:], lg[:], m1[:].to_broadcast(bc),
                            op=mybir.AluOpType.subtract)
    mask1 = small.tile([P, NCHUNK, E], F32)
    nc.vector.tensor_scalar(mask1[:], ls[:], 0.0, None,
                            op0=mybir.AluOpType.is_ge)
    masked = small.tile([P, NCHUNK, E], F32)
    nc.vector.scalar_tensor_tensor(out=masked[:], in0=mask1[:], scalar=-1e30,
                                   in1=ls[:], op0=mybir.AluOpType.mult,
                                   op1=mybir.AluOpType.add)
    m2 = small.tile([P, NCHUNK, 1], F32)
    nc.vector.reduce_max(m2[:], masked[:], axis=mybir.AxisListType.X)
    mask12 = small.tile([P, NCHUNK, E], F32)
    nc.vector.tensor_tensor(mask12[:], ls[:], m2[:].to_broadcast(bc),
                            op=mybir.AluOpType.is_ge)
    ex = small.tile([P, NCHUNK, E], F32)
    nc.scalar.activation(ex[:], ls[:], mybir.ActivationFunctionType.Exp)
    wun = small.tile([P, NCHUNK, E], F32)
    nc.vector.tensor_tensor(wun[:], ex[:], mask12[:], op=mybir.AluOpType.mult)
    den = small.tile([P, NCHUNK, 1], F32)
    nc.vector.reduce_sum(den[:], wun[:], axis=mybir.AxisListType.X)
    rden = small.tile([P, NCHUNK, 1], F32)
    nc.vector.reciprocal(rden[:], den[:])
    cw = small.tile([P, NCHUNK, E], F32)
    nc.vector.tensor_tensor(cw[:], wun[:],
                            esel_s[:].unsqueeze(1).to_broadcast(bc),
                            op=mybir.AluOpType.mult)
    cwn = small.tile([P, NCHUNK, E], F32)
    nc.vector.tensor_tensor(cwn[:], cw[:], rden[:].to_broadcast(bc),
                            op=mybir.AluOpType.mult)
    c_e = small.tile([P, NCHUNK, 1], F32)
    nc.vector.reduce_sum(c_e[:], cwn[:], axis=mybir.AxisListType.X)
    return c_e


def _phase_a(nc, psA, wpool, evac, xt_s, inter, w1v, w3v, t0):
    """interT[f, t] = silu(w1.T x) * (w3.T x) for tokens [t0, t0+TH)."""
    for f in range(KF):
        w1f = wpool.tile([P, KH, P], F32R, tag="w1f", name="w1f", bufs=3)
        nc.gpsimd.dma_start(out=w1f[:], in_=w1v[:, :, f * P:(f + 1) * P])
        w3f = wpool.tile([P, KH, P], F32R, tag="w3f", name="w3f", bufs=3)
        nc.gpsimd.dma_start(out=w3f[:], in_=w3v[:, :, f * P:(f + 1) * P])
        for n in range(TH // 512):
            xtn = xt_s[(t0 + n * 512) // 512]
            fs = slice(n * 512, (n + 1) * 512)
            ps1 = psA.tile([P, 512], F32, tag="ps1", name="ps1")
            for k in range(KH):
                nc.tensor.matmul(ps1[:], lhsT=w1f[:, k, :], rhs=xtn[:, k, :],
                                 start=(k == 0), stop=(k == KH - 1))
            ps3 = psA.tile([P, 512], F32, tag="ps3", name="ps3")
            for k in range(KH):
                nc.tensor.matmul(ps3[:], lhsT=w3f[:, k, :], rhs=xtn[:, k, :],
                                 start=(k == 0), stop=(k == KH - 1))
            sil = evac.tile([P, 512], F32, tag="sil", name="sil")
            nc.scalar.activation(sil[:], ps1[:],
                                 mybir.ActivationFunctionType.Silu)
            nc.vector.tensor_tensor(inter[:, f, fs], sil[:], ps3[:],
                                    op=mybir.AluOpType.mult)


def _phase_b(nc, psB, wpool, evac, inter, w2v, c_e, cc_q, th):
    """cc_q[g][t, :] = (interT.T @ w2) * c_e for this T-half's quarters."""
    for g in range(MH // MG):
        cc_in = cc_q[g]
        psbs = [[psB.tile([P, 512], F32, tag=f"psb{m}{n}", name=f"psb{m}{n}")
                 for n in range(H // 512)] for m in range(MG)]
        for k in range(KF):
            w2k = wpool.tile([P, H], F32R, tag="w2k", name="w2k", bufs=4)
            nc.gpsimd.dma_start(out=w2k[:], in_=w2v[:, k, :])
            for m in range(MG):
                ma = g * MG + m
                for n in range(H // 512):
                    nc.tensor.matmul(psbs[m][n][:],
                                     lhsT=inter[:, k, ma * P:(ma + 1) * P],
                                     rhs=w2k[:, n * 512:(n + 1) * 512],
                                     start=(k == 0), stop=(k == KF - 1))
        for m in range(MG):
            ma = g * MG + m
            for n in range(H // 512):
                o = evac.tile([P, 512], F32, tag="o", name="o")
                nc.vector.tensor_scalar_mul(o[:], psbs[m][n][:],
                                            c_e[:, th * MH + ma, :])
                nc.sync.dma_start(
                    out=cc_in.ap()[m * P:(m + 1) * P,
                                   n * 512:(n + 1) * 512],
                    in_=o[:])


def build():
    nc = bacc.Bacc("TRN2", target_bir_lowering=False, debug=False,
                   num_devices=E)
    xt = nc.dram_tensor("xt", [H, T], F32R, kind="ExternalInput")
    gw = nc.dram_tensor("gw", [H, E], F32R, kind="ExternalInput")
    esel = nc.dram_tensor("esel", [P, E], F32, kind="ExternalInput")
    w1 = nc.dram_tensor("w1", [H, F], F32R, kind="ExternalInput")
    w3 = nc.dram_tensor("w3", [H, F], F32R, kind="ExternalInput")
    w2 = nc.dram_tensor("w2", [F, H], F32R, kind="ExternalInput")
    out_shard = nc.dram_tensor("out_shard", [2 * P, H], F32,
                               kind="ExternalOutput")

    NQ = 4
    TQ = T // NQ  # 512 tokens per quarter
    cc_in = [nc.dram_tensor(f"cc_in{i}", [TQ, H], F32, kind="Internal")
             for i in range(NQ)]
    cc_out = [nc.dram_tensor(f"cc_out{i}", [TQ // E, H], F32, kind="Internal")
              for i in range(NQ)]

    with tile.TileContext(nc) as tc:
        with (
            tc.tile_pool(name="big", bufs=1) as big,
            tc.tile_pool(name="small", bufs=1) as small,
            tc.tile_pool(name="wpool", bufs=2) as wpool,
            tc.tile_pool(name="evac", bufs=4) as evac,
        ):
            xtv = xt.ap().rearrange("(k p) t -> p k t", p=P)
            xt_s = []
            for n in range(T // 512):  # separate tiles so compute starts early
                xtn = big.tile([P, KH, 512], F32R, name=f"xt{n}")
                eng = nc.gpsimd if n == 0 else nc.sync
                eng.dma_start(out=xtn[:],
                              in_=xtv[:, :, n * 512:(n + 1) * 512])
                xt_s.append(xtn)
            inter = big.tile([P, KF, TH], F32R)  # interT for current half

            gw_s = small.tile([P, KH, E], F32R)
            nc.sync.dma_start(out=gw_s[:],
                              in_=gw.ap().rearrange("(k p) e -> p k e", p=P))
            esel_s = small.tile([P, E], F32)
            nc.sync.dma_start(out=esel_s[:], in_=esel.ap())
            ident = small.tile([P, P], F32)
            make_identity(nc, ident[:])

            w1v = w1.ap().rearrange("(k p) f -> p k f", p=P)
            w3v = w3.ap().rearrange("(k p) f -> p k f", p=P)
            w2v = w2.ap().rearrange("(k p) h -> p k h", p=P)

            # half 0 phase A first so PE starts as soon as xt chunk 0 lands
            with tc.tile_pool(name="psA0", bufs=2, space=PSUM) as psA:
                _phase_a(nc, psA, wpool, evac, xt_s, inter, w1v, w3v, 0)
            c_e = _router(nc, tc, small, xt_s, gw_s, esel_s, ident)
            def rs(q):
                nc.gpsimd.collective_compute(
                    "ReduceScatter", mybir.AluOpType.add,
                    replica_groups=[list(range(E))],
                    ins=[cc_in[q].ap()], outs=[cc_out[q].ap()])

            with tc.tile_pool(name="psB0", bufs=1, space=PSUM) as psB:
                _phase_b(nc, psB, wpool, evac, inter, w2v, c_e, cc_in[0:2], 0)
            rs(0)
            with tc.tile_pool(name="psA1", bufs=2, space=PSUM) as psA:
                _phase_a(nc, psA, wpool, evac, xt_s, inter, w1v, w3v, TH)
            rs(1)
            with tc.tile_pool(name="psB1", bufs=1, space=PSUM) as psB:
                _phase_b(nc, psB, wpool, evac, inter, w2v, c_e, cc_in[2:4], 1)
            rs(2)
            rs(3)

            TQ8 = (T // 4) // E
            for q in range(4):
                nc.sync.dma_start(
                    out=out_shard.ap()[q * TQ8:(q + 1) * TQ8, :],
                    in_=cc_out[q].ap())
    nc.compile()
    return nc


def kernel(hidden_states, gate_w, w1, w2, w3):
    if "nc" not in _NC_CACHE:
        _NC_CACHE["nc"] = build()
    nc = _NC_CACHE["nc"]

    res = run_bass_kernel_spmd(nc, make_in_maps(hidden_states, gate_w, w1, w2, w3),
                               core_ids=list(range(E)), trace=False)
    return assemble(res.results)


def make_in_maps(hidden_states, gate_w, w1, w2, w3):
    xt = np.ascontiguousarray(hidden_states.T)
    in_maps = []
    for e in range(E):
        sel = np.zeros((P, E), dtype=np.float32)
        sel[:, e] = 1.0
        in_maps.append({
            "xt": xt,
            "gw": np.ascontiguousarray(gate_w),
            "esel": sel,
            "w1": np.ascontiguousarray(w1[e]),
            "w3": np.ascontiguousarray(w3[e]),
            "w2": np.ascontiguousarray(w2[e]),
        })
    return in_maps


def assemble(results):
    out = np.empty((T, H), dtype=np.float32)
    tq = T // 4
    rq = tq // E  # 64 rows per core per quarter
    for r in range(E):
        sh = results[r]["out_shard"]
        for q in range(4):
            t0 = q * tq + r * rq
            out[t0:t0 + rq] = sh[q * rq:(q + 1) * rq]
    return out
